# revision 8
# baseline (speedup 1.0000x reference)
"""Trainium2 Bass kernel for the fused sparse-attention block.

Computes (8-core SPMD, head-parallel + final row-shard re-layout):
    qkv = x @ W_qkv; q,k = rope(rmsnorm(q|k)); causal attention;
    out = (attn_out * sigmoid(x @ W_gate + b_gate)) @ W_out

Per core c (heads 2c, 2c+1 for both batches):
  Phase 1: XBAR DMA-transpose x chunks; v/q/k projections in bf16
           (16 matmuls per coltile), gate projection in fp8e4 DoubleRow
           (8 matmuls, weights pre-scaled x64 host-side; xt converted to
           paired-fp8 layout by DVE copies).  RMSNorm via all-ones f32r
           reduce matmul, RoPE swap on gpsimd.  Gate preactivation parked
           raw; tanh sweep happens in phase 2 under the exp table set
           (sigmoid(z) == (1+tanh(z/2))/2, the 0.5 folded into W_out).
  Phase 2: one flat software-pipelined stream over all (head,batch,qc,kt)
           tiles: scores into 2-bank [128,1024] PSUM pair-tiles, one Exp
           per pair, causal 0/1 mask per diagonal pair, PV accumulation
           lagging LOOK slots behind scores; denominator accumulated on
           DVE into acc2 and reduced by two f32r all-ones matmuls; gate
           applied as (tanh+1)*nm via scalar_tensor_tensor.
  Phase 3: one AllToAll per head (head-shard -> row-shard) with gat loads
           issued right after each collective on the sync queue;
           row-sharded output projection with full W_out (pre-scaled 0.5).
"""
import sys
if '/opt/trn_rl_repo' not in sys.path:
    sys.path.insert(0, '/opt/trn_rl_repo')

import numpy as np


def _install_ntff_hook_shim():
    """Provide antenv.axon_hooks if the image lacks it (needed only when a
    caller requests NTFF tracing through run_bass_kernel_spmd)."""
    import types
    if 'antenv.axon_hooks' in sys.modules:
        return
    try:
        import antenv
    except ImportError:
        return
    if hasattr(antenv, 'axon_hooks'):
        return
    mod = types.ModuleType('antenv.axon_hooks')
    _state = {}

    def set_axon_ntff_profile_hook(h):
        _state['hook'] = h

    def get_axon_ntff_profile_hook():
        if 'hook' not in _state:
            try:
                from trn_agent_boot.trn_boot import _ntff_profile_via_ctypes
                _state['hook'] = _ntff_profile_via_ctypes('/opt/axon/libaxon_pjrt.so')
            except Exception:
                _state['hook'] = None
        return _state['hook']

    mod.set_axon_ntff_profile_hook = set_axon_ntff_profile_hook
    mod.get_axon_ntff_profile_hook = get_axon_ntff_profile_hook
    sys.modules['antenv.axon_hooks'] = mod
    antenv.axon_hooks = mod


_install_ntff_hook_shim()

B, T, D = 2, 2048, 2048
H = 16
d = 128
N_CORES = 8
HPC = H // N_CORES          # heads per core = 2
ROWS = B * T                # 4096
RC = 512                    # rows per phase-1 chunk
NRC = ROWS // RC            # 8 row chunks
KC = D // 128               # 16 contraction chunks
BF_CT = 6                   # bf16 coltiles: v0 v1 q0 q1 k0 k1
QCH = 512                   # attention q chunk
LOOK = 6                    # kt slots of score->pv lookahead
EPS = 1e-6
ROPE_BASE = 10000.0
SCALE = 1.0 / np.sqrt(d)
GSC = 64.0                  # gate fp8 weight prescale

_cache = {}


def _build():
    import concourse.bacc as bacc
    import concourse.mybir as mybir
    from concourse.tile import TileContext

    f32 = mybir.dt.float32
    f32r = mybir.dt.float32r
    bf16 = mybir.dt.bfloat16
    f8 = mybir.dt.float8e4
    AF = mybir.ActivationFunctionType
    DR = mybir.MatmulPerfMode.DoubleRow

    def r_(ap):
        return ap.bitcast(f32r)

    nc = bacc.Bacc("TRN2", target_bir_lowering=False, debug=False,
                   num_devices=N_CORES)

    x_in = nc.dram_tensor("x", [ROWS, D], bf16, kind="ExternalInput").ap()
    w_in = nc.dram_tensor("w_qkv6", [D, BF_CT * 128], bf16,
                          kind="ExternalInput").ap()
    w8g_in = nc.dram_tensor("w8g", [128, HPC * 8 * 2 * 128], f8,
                            kind="ExternalInput").ap()
    wout_in = nc.dram_tensor("w_out", [D, D], bf16, kind="ExternalInput").ap()
    bg_in = nc.dram_tensor("b_gate", [128, HPC], f32, kind="ExternalInput").ap()
    cos_in = nc.dram_tensor("costab", [128, T], f32, kind="ExternalInput").ap()
    sin_in = nc.dram_tensor("sintab", [128, T], f32, kind="ExternalInput").ap()
    mask_in = nc.dram_tensor("m01", [128, 4 * QCH], bf16, kind="ExternalInput").ap()
    out_ext = nc.dram_tensor("out", [RC, D], bf16, kind="ExternalOutput").ap()

    with TileContext(nc) as tc:
        with tc.tile_pool(name="persist", bufs=1) as persist, \
             tc.tile_pool(name="dram", bufs=1, space="DRAM") as dram:
            a2a_in = [dram.tile([N_CORES * 128, RC], bf16, name=f"a2a_in{h}")
                      for h in range(HPC)]
            a2a_out = [dram.tile([N_CORES * 128, RC], bf16, name=f"a2a_out{h}")
                       for h in range(HPC)]

            ones_sq = persist.tile([128, 128], bf16, tag="ones_sq")
            ones_sqf = persist.tile([128, 128], f32, tag="ones_sqf")
            eps_col = persist.tile([128, 1], f32, tag="eps")
            bg_sb = persist.tile([128, HPC], f32, tag="bg")
            mask_sb = persist.tile([128, 4 * QCH], bf16, tag="mask")
            cc_sb = persist.tile([128, T], f32, tag="cc")
            ss_sb = persist.tile([128, T], f32, tag="ss")
            qT = [[persist.tile([128, T], bf16, tag=f"qT{h}{b}", name=f"qT{h}{b}")
                   for b in range(B)] for h in range(HPC)]
            kT = [[persist.tile([128, T], bf16, tag=f"kT{h}{b}", name=f"kT{h}{b}")
                   for b in range(B)] for h in range(HPC)]
            gT = [[persist.tile([128, T], bf16, tag=f"gT{h}{b}", name=f"gT{h}{b}")
                   for b in range(B)] for h in range(HPC)]
            v_sb = [persist.tile([128, ROWS], bf16, tag=f"v{h}", name=f"v_sb{h}")
                    for h in range(HPC)]

            # ---------------- Phase 1 ----------------
            with tc.tile_pool(name="wq", bufs=1) as wq, \
                 tc.tile_pool(name="p1", bufs=2) as p1, \
                 tc.tile_pool(name="p1xt", bufs=2) as p1xt, \
                 tc.tile_pool(name="p1x8", bufs=2) as p1x8, \
                 tc.tile_pool(name="pp_pj", bufs=6, space="PSUM") as pp_pj, \
                 tc.tile_pool(name="pp_st", bufs=2, space="PSUM") as pp_st:
                # chunk-0 x transpose first (sync queue owns all xbar
                # transposes: each fans out over all 16 DMA engines, two in
                # flight interleave and corrupt); weights on the scalar
                # HWDGE queue so they don't delay the transpose.
                xt0 = p1xt.tile([128, KC, RC], bf16, tag="xt", name="xt_0")
                nc.sync.dma_start(out=xt0[:], in_=x_in[0:RC, :], transpose=True)
                # w_sb[p, k, ct*128+c] = W[k*128+p, ct*128+c]; 4 k-group DMAs
                w_sb = wq.tile([128, KC, BF_CT * 128], bf16, tag="w")
                for kg in range(4):
                    nc.scalar.dma_start(
                        out=w_sb[:, kg * 4:(kg + 1) * 4, :],
                        in_=w_in[kg * 512:(kg + 1) * 512, :].rearrange(
                            "(k p) c -> p k c", p=128))
                w8g_sb = wq.tile([128, HPC, 8, 2, 128], f8, tag="w8g")
                nc.scalar.dma_start(
                    out=w8g_sb[:],
                    in_=w8g_in[:].rearrange("p (h k j m) -> p h k j m",
                                            h=HPC, k=8, j=2))
                nc.scalar.dma_start(out=cc_sb[:], in_=cos_in[:])
                nc.scalar.dma_start(out=ss_sb[:], in_=sin_in[:])
                nc.scalar.dma_start(out=mask_sb[:], in_=mask_in[:])
                nc.scalar.dma_start(out=bg_sb[:], in_=bg_in[:])
                nc.vector.memset(ones_sq[:], 1.0)
                nc.vector.memset(ones_sqf[:], 1.0)
                nc.vector.memset(eps_col[:], EPS)

                for rc in range(NRC):
                    b = rc // 4
                    t0 = (rc % 4) * RC

                    if rc == 0:
                        xt = xt0
                    else:
                        xt = p1xt.tile([128, KC, RC], bf16, tag="xt",
                                       name=f"xt{rc}")
                        nc.sync.dma_start(
                            out=xt[:],
                            in_=x_in[rc * RC:(rc + 1) * RC, :],
                            transpose=True)
                    # paired-fp8 copy of xt for the gate DoubleRow matmuls
                    xt8 = p1x8.tile([128, 8, 2, RC], f8, tag="xt8",
                                    name=f"xt8_{rc}")
                    for k8 in range(8):
                        for j in range(2):
                            nc.vector.tensor_copy(xt8[:, k8, j, :],
                                                  xt[:, 2 * k8 + j, :])

                    # bf16 coltiles: 0=v0 1=v1 2=q0 3=q1 4=k0 5=k1
                    for ct in range(BF_CT):
                        h = ct % 2
                        ps = pp_pj.tile([128, RC], f32, tag="pj",
                                        name=f"pj{rc}_{ct}")
                        for k in range(KC):
                            nc.tensor.matmul(
                                ps[:],
                                w_sb[:, k, ct * 128:(ct + 1) * 128],
                                xt[:, k, :],
                                start=(k == 0), stop=(k == KC - 1))
                        if ct < 2:
                            # v: evict bf16, re-layout to natural via XBAR DMA
                            sv = p1.tile([128, RC], bf16, tag="sv")
                            nc.scalar.activation(sv[:], ps[:], AF.Copy)
                            vview = v_sb[h][:].rearrange(
                                "p (rt dd) -> p rt dd", dd=128)
                            nc.sync.dma_start(
                                out=vview[:, rc * 4:(rc + 1) * 4, :],
                                in_=sv[:], transpose=True)
                        else:
                            # q or k head: rmsnorm (all-ones f32r reduce
                            # pre-broadcasts across partitions) + rope
                            isq = ct < 4
                            sq = p1.tile([128, RC], f32r, tag="sq")
                            nc.scalar.activation(sq[:], ps[:], AF.Square)
                            ssq = pp_st.tile([128, RC], f32, tag="ssq")
                            nc.tensor.matmul(ssq[:], r_(ones_sqf[:]), sq[:],
                                             start=True, stop=True)
                            bc_sb = p1.tile([128, RC], f32, tag="bc_sb")
                            nc.scalar.activation(bc_sb[:], ssq[:],
                                                 AF.Abs_reciprocal_sqrt,
                                                 scale=1.0 / 128.0,
                                                 bias=eps_col[:])
                            qn = p1.tile([128, RC], f32, tag="qn")
                            nc.vector.tensor_mul(qn[:], ps[:], bc_sb[:])
                            # rope: fin = qn*cc + swap(qn)*ss; partition swap
                            # must be a copy (tensor_tensor ops need equal
                            # start partitions); cc=[cos;cos], ss=[-sin;sin]
                            sw = p1.tile([128, RC], f32, tag="sw")
                            nc.gpsimd.tensor_copy(sw[0:64, :], qn[64:128, :])
                            nc.gpsimd.tensor_copy(sw[64:128, :], qn[0:64, :])
                            nc.vector.tensor_mul(sw[:], sw[:], ss_sb[:, t0:t0 + RC])
                            nc.vector.tensor_mul(qn[:], qn[:], cc_sb[:, t0:t0 + RC])
                            park = qT if isq else kT
                            nc.vector.tensor_add(
                                park[h][b][:, t0:t0 + RC], qn[:], sw[:])
                    # gate coltiles in fp8 DoubleRow (weights x64 host-side)
                    for h in range(HPC):
                        ps = pp_pj.tile([128, RC], f32, tag="pj",
                                        name=f"pjg{rc}_{h}")
                        for k8 in range(8):
                            nc.tensor.matmul(
                                ps[:], w8g_sb[:, h, k8, :, :],
                                xt8[:, k8, :, :],
                                start=(k8 == 0), stop=(k8 == 7),
                                perf_mode=DR)
                        # park raw preactivation (x64); tanh sweep in phase 2
                        nc.scalar.activation(gT[h][b][:, t0:t0 + RC], ps[:],
                                             AF.Copy)

            # ---------------- Phase 2 (flat pipelined) + wout prefetch ----
            with tc.tile_pool(name="wout", bufs=1) as woutp:
                wout_tiles = {}
                for oc in range(4):
                    wt = woutp.tile([128, KC, 512], bf16, tag=f"wo{oc}",
                                    name=f"wo{oc}")
                    for kg in range(2):
                        nc.scalar.dma_start(
                            out=wt[:, kg * 8:(kg + 1) * 8, :],
                            in_=wout_in[kg * 1024:(kg + 1) * 1024,
                                        oc * 512:(oc + 1) * 512].rearrange(
                                            "(k p) c -> p k c", p=128))
                    wout_tiles[oc] = wt
                # gat[p, k, r]: k even = head-0 dims (A2A#1), k odd = head-1
                # (A2A#2); loads issued right after each collective below
                gat = woutp.tile([128, KC, RC], bf16, tag="gat")

                PAIRS = ((0, 0), (0, 1), (1, 0), (1, 1))
                # flattened (h, b, qc, kt) score slots
                slots = []
                for h, bb in PAIRS:
                    for qc in range(T // QCH):
                        nkt = 4 * qc + 4
                        for kt in range(nkt):
                            slots.append((h, bb, qc, kt, nkt))
                NS = len(slots)

                with tc.tile_pool(name="p2", bufs=2) as p2, \
                     tc.tile_pool(name="p2e", bufs=6) as p2e, \
                     tc.tile_pool(name="p2a", bufs=2) as p2a, \
                     tc.tile_pool(name="pp_s", bufs=2, space="PSUM") as pp_s, \
                     tc.tile_pool(name="pp_o", bufs=2, space="PSUM") as pp_o, \
                     tc.tile_pool(name="pp_d", bufs=1, space="PSUM") as pp_d:
                    sc2 = {}      # pair index -> psum tile
                    ex2 = {}      # pair index -> sbuf exp tile
                    o_ps = {}     # (h,b,qc) -> pv psum tile
                    acc2 = {}     # (h,b,qc) -> dve denominator accumulator
                    swept = set()

                    def j0_of(qc, kt):
                        m = kt - 4 * qc
                        return 128 * m if m > 0 else 0

                    for i in range(NS + LOOK):
                        if i < NS:
                            h, bb, qc, kt, nkt = slots[i]
                            col0 = qc * QCH
                            qmv = qT[h][bb][:, col0:col0 + QCH]
                            j0 = j0_of(qc, kt)
                            half = kt % 2
                            pidx = i // 2
                            if half == 0:
                                sc2[pidx] = pp_s.tile([128, 2 * QCH], f32,
                                                      tag="sc",
                                                      name=f"sc{pidx}")
                            nc.tensor.matmul(
                                sc2[pidx][:, half * QCH + j0:(half + 1) * QCH],
                                kT[h][bb][:, kt * 128:(kt + 1) * 128],
                                qmv[:, j0:], start=True, stop=True)
                            if half == 1:
                                # exp over the whole pair; the region left of
                                # a diagonal j0 holds stale-but-finite psum,
                                # zeroed by the mask multiply below
                                j0e = j0_of(qc, kt - 1)
                                ex = p2e.tile([128, 2 * QCH], bf16, tag="ex",
                                              name=f"ex{pidx}")
                                nc.scalar.activation(ex[:, j0e:], sc2[pidx][:, j0e:],
                                                     AF.Exp, scale=SCALE)
                                m_e = (kt - 1) - 4 * qc
                                if m_e >= 0:
                                    nc.vector.tensor_mul(
                                        ex[:, j0e:], ex[:, j0e:],
                                        mask_sb[:, m_e * QCH + j0e:
                                                (m_e + 2) * QCH])
                                ex2[pidx] = ex
                                key = (h, bb, qc)
                                if kt == 1:
                                    a = p2a.tile([128, 2 * QCH], f32r, tag="acc",
                                                 name=f"acc{h}{bb}{qc}")
                                    nc.vector.tensor_copy(a[:], ex[:])
                                    acc2[key] = a
                                else:
                                    nc.vector.tensor_add(
                                        acc2[key][:, j0e:], acc2[key][:, j0e:],
                                        ex[:, j0e:])
                                # tanh sweep for this pair's gate, under the
                                # exp table set, right after the first exp
                                if (h, bb) not in swept:
                                    swept.add((h, bb))
                                    nc.scalar.activation(
                                        gT[h][bb][:], gT[h][bb][:], AF.Tanh,
                                        scale=1.0 / (2.0 * GSC),
                                        bias=bg_sb[:, h:h + 1])
                        if i >= LOOK:
                            h, bb, qc, kt, nkt = slots[i - LOOK]
                            col0 = qc * QCH
                            j0 = j0_of(qc, kt)
                            half = kt % 2
                            pidx = (i - LOOK) // 2
                            key = (h, bb, qc)
                            if kt == 0:
                                o_ps[key] = pp_o.tile([128, QCH], f32, tag="o",
                                                      name=f"o{h}{bb}{qc}")
                            nc.tensor.matmul(
                                o_ps[key][:, j0:],
                                v_sb[h][:, (bb * 16 + kt) * 128:
                                        (bb * 16 + kt + 1) * 128],
                                ex2[pidx][:, half * QCH + j0:(half + 1) * QCH],
                                start=(kt == 0), stop=(kt == nkt - 1),
                                skip_group_check=(j0 > 0))
                            if kt == nkt - 1:
                                # denominator: two f32r all-ones reduces into
                                # the halves of a 2-bank psum tile, then merge
                                a = acc2.pop(key)
                                den = pp_d.tile([128, QCH], f32, tag="den",
                                                name=f"den{h}{bb}{qc}")
                                nc.tensor.matmul(den[:], r_(ones_sqf[:]),
                                                 a[:, 0:QCH],
                                                 start=True, stop=False)
                                nc.tensor.matmul(den[:], r_(ones_sqf[:]),
                                                 a[:, QCH:],
                                                 start=False, stop=True)
                                rec = p2.tile([128, QCH], f32, tag="rec")
                                nc.vector.reciprocal_approx_fast(rec[:], den[:])
                                nm = p2.tile([128, QCH], f32, tag="nm")
                                nc.vector.tensor_mul(nm[:], o_ps.pop(key)[:],
                                                     rec[:])
                                # gate: sigmoid(z) = (1+tanh(z/2))/2, the 0.5
                                # lives in W_out; stt: (gT + 1) * nm
                                on_sb = p2.tile([128, QCH], bf16, tag="onsb")
                                nc.vector.scalar_tensor_tensor(
                                    out=on_sb[:],
                                    in0=gT[h][bb][:, col0:col0 + QCH],
                                    scalar=1.0,
                                    in1=nm[:],
                                    op0=mybir.AluOpType.add,
                                    op1=mybir.AluOpType.mult)
                                shard = bb * 4 + qc
                                nc.sync.dma_start(
                                    out=a2a_in[h][shard * 128:(shard + 1) * 128, :],
                                    in_=on_sb[:])
                                if bb == 1 and qc == T // QCH - 1:
                                    nc.gpsimd.collective_compute(
                                        "AllToAll", mybir.AluOpType.bypass,
                                        replica_groups=[list(range(N_CORES))],
                                        ins=[a2a_in[h].opt()],
                                        outs=[a2a_out[h].opt()])
                                    nc.sync.dma_start(
                                        out=gat[:, h:KC:2, :],
                                        in_=a2a_out[h][:].rearrange(
                                            "(k p) c -> p k c", p=128))

                # ---------------- Phase 3: output projection ----------------
                with tc.tile_pool(name="p3", bufs=1) as p3, \
                     tc.tile_pool(name="p3e", bufs=3) as p3e, \
                     tc.tile_pool(name="pp_3", bufs=8, space="PSUM") as pp_3:
                    # pass A: even k (ready after A2A#1) -> SBUF partials
                    partials = {}
                    for oc in range(4):
                        for rt in range(4):
                            ps = pp_3.tile([128, 512], f32, tag="o3",
                                           name=f"psA_{oc}_{rt}")
                            for k in range(0, KC, 2):
                                nc.tensor.matmul(
                                    ps[:], gat[:, k, rt * 128:(rt + 1) * 128],
                                    wout_tiles[oc][:, k, :],
                                    start=(k == 0), stop=(k == KC - 2))
                            pa = p3.tile([128, 512], bf16, tag=f"pa{oc}{rt}",
                                         name=f"pa_{oc}_{rt}")
                            nc.vector.tensor_copy(pa[:], ps[:])
                            partials[(oc, rt)] = pa
                    # pass B: odd k (after A2A#2), add partial at evict
                    for oc in range(4):
                        for rt in range(4):
                            ps = pp_3.tile([128, 512], f32, tag="o3",
                                           name=f"psB_{oc}_{rt}")
                            for k in range(1, KC, 2):
                                nc.tensor.matmul(
                                    ps[:], gat[:, k, rt * 128:(rt + 1) * 128],
                                    wout_tiles[oc][:, k, :],
                                    start=(k == 1), stop=(k == KC - 1))
                            ev = p3e.tile([128, 512], bf16, tag="ev")
                            nc.vector.tensor_add(ev[:], ps[:], partials[(oc, rt)][:])
                            nc.sync.dma_start(
                                out=out_ext[rt * 128:(rt + 1) * 128,
                                            oc * 512:(oc + 1) * 512],
                                in_=ev[:])

    nc.compile()
    return nc


def _tables():
    inv = 1.0 / (ROPE_BASE ** (np.arange(0, d, 2, dtype=np.float64) / d))
    pos = np.arange(T, dtype=np.float64)
    ang = pos[None, :] * inv[:, None]          # [64, T]
    cos = np.cos(ang).astype(np.float32)
    sin = np.sin(ang).astype(np.float32)
    cc = np.concatenate([cos, cos], axis=0)    # [128, T]
    ss = np.concatenate([-sin, sin], axis=0)   # [128, T]
    return cc, ss


def kernel(x, W_qkv, W_out, W_gate, b_gate, mask):
    from concourse.bass_utils import run_bass_kernel_spmd
    import ml_dtypes

    if 'nc' not in _cache:
        _cache['nc'] = _build()
    nc = _cache['nc']

    x = np.ascontiguousarray(
        np.asarray(x, dtype=np.float32).reshape(ROWS, D)).astype(ml_dtypes.bfloat16)
    W_qkv = np.asarray(W_qkv, dtype=np.float32)
    W_out = np.ascontiguousarray(
        0.5 * np.asarray(W_out, dtype=np.float32)).astype(ml_dtypes.bfloat16)
    W_gate = np.asarray(W_gate, dtype=np.float32)
    b_gate = np.asarray(b_gate, dtype=np.float32)
    cos, sin = _tables()
    f = np.arange(QCH)[None, :]
    p = np.arange(128)[:, None]
    m01 = np.concatenate(
        [np.where(f >= p + 128 * m, 1.0, 0.0) for m in range(4)],
        axis=1).astype(ml_dtypes.bfloat16)

    in_maps = []
    for c in range(N_CORES):
        h0 = HPC * c
        cols = []
        for kind in (2, 0, 1):                    # v, q, k coltiles
            for h in range(h0, h0 + HPC):
                cols.append(W_qkv[:, kind * D + h * d:(kind * D + (h + 1) * d)])
        w_qkv6 = np.ascontiguousarray(
            np.concatenate(cols, axis=1)).astype(ml_dtypes.bfloat16)
        # gate fp8 DoubleRow tiles: w8g[p, h, k8, j, m] =
        #   GSC * W_gate[(2*k8+j)*128 + p, (h0+h)*128 + m]
        wg = W_gate[:, h0 * d:(h0 + HPC) * d] * GSC       # [2048, 256]
        w8 = wg.reshape(8, 2, 128, HPC, 128)              # [k8, j, p, h, m]
        w8 = w8.transpose(2, 3, 0, 1, 4).reshape(128, HPC * 8 * 2 * 128)
        w8g = np.ascontiguousarray(w8).astype(ml_dtypes.float8_e4m3)
        bg = np.ascontiguousarray(
            0.5 * b_gate[h0 * d:(h0 + HPC) * d].reshape(HPC, 128).T)
        in_maps.append({
            "x": x, "w_qkv6": w_qkv6, "w8g": w8g, "w_out": W_out,
            "b_gate": bg, "costab": cos, "sintab": sin, "m01": m01,
        })

    res = run_bass_kernel_spmd(nc, in_maps, list(range(N_CORES)))
    _cache['last_results'] = res
    out = np.concatenate(
        [np.asarray(res.results[c]["out"]).astype(np.float32)
         for c in range(N_CORES)], axis=0)
    return out.reshape(B, T, D)


# revision 10
# speedup vs baseline: 1.0278x; 1.0278x over previous
"""Trainium2 Bass kernel for the fused sparse-attention block.

Computes (8-core SPMD, head-parallel + final row-shard re-layout):
    qkv = x @ W_qkv; q,k = rope(rmsnorm(q|k)); causal attention;
    out = (attn_out * sigmoid(x @ W_gate + b_gate)) @ W_out

Per core c (heads 2c, 2c+1 for both batches):
  Phase 1: XBAR DMA-transpose x chunks; v/q/k projections in bf16
           (16 matmuls per coltile), gate projection in fp8e4 DoubleRow
           (8 matmuls, weights pre-scaled x64 host-side; xt converted to
           paired-fp8 layout by DVE copies).  RMSNorm via all-ones f32r
           reduce matmul, RoPE swap on gpsimd.  Gate preactivation parked
           raw; tanh sweep happens in phase 2 under the exp table set
           (sigmoid(z) == (1+tanh(z/2))/2, the 0.5 folded into W_out).
  Phase 2: one flat software-pipelined stream over all (head,batch,qc,kt)
           tiles: scores into 2-bank [128,1024] PSUM pair-tiles, one Exp
           per pair, causal 0/1 mask per diagonal pair, PV accumulation
           lagging LOOK slots behind scores; denominator accumulated on
           DVE into acc2 and reduced by two f32r all-ones matmuls; gate
           applied as (tanh+1)*nm via scalar_tensor_tensor.
  Phase 3: one AllToAll per head (head-shard -> row-shard) with gat loads
           issued right after each collective on the sync queue;
           row-sharded output projection with full W_out (pre-scaled 0.5).
"""
import sys
if '/opt/trn_rl_repo' not in sys.path:
    sys.path.insert(0, '/opt/trn_rl_repo')

import numpy as np


def _install_ntff_hook_shim():
    """Provide antenv.axon_hooks if the image lacks it (needed only when a
    caller requests NTFF tracing through run_bass_kernel_spmd)."""
    import types
    if 'antenv.axon_hooks' in sys.modules:
        return
    try:
        import antenv
    except ImportError:
        return
    if hasattr(antenv, 'axon_hooks'):
        return
    mod = types.ModuleType('antenv.axon_hooks')
    _state = {}

    def set_axon_ntff_profile_hook(h):
        _state['hook'] = h

    def get_axon_ntff_profile_hook():
        if 'hook' not in _state:
            try:
                from trn_agent_boot.trn_boot import _ntff_profile_via_ctypes
                _state['hook'] = _ntff_profile_via_ctypes('/opt/axon/libaxon_pjrt.so')
            except Exception:
                _state['hook'] = None
        return _state['hook']

    mod.set_axon_ntff_profile_hook = set_axon_ntff_profile_hook
    mod.get_axon_ntff_profile_hook = get_axon_ntff_profile_hook
    sys.modules['antenv.axon_hooks'] = mod
    antenv.axon_hooks = mod


_install_ntff_hook_shim()

B, T, D = 2, 2048, 2048
H = 16
d = 128
N_CORES = 8
HPC = H // N_CORES          # heads per core = 2
ROWS = B * T                # 4096
RC = 512                    # rows per phase-1 chunk
NRC = ROWS // RC            # 8 row chunks
KC = D // 128               # 16 contraction chunks
BF_CT = 6                   # bf16 coltiles: v0 v1 q0 q1 k0 k1
QCH = 512                   # attention q chunk
LOOK = 4                    # kt slots of score->pv lookahead
EPS = 1e-6
ROPE_BASE = 10000.0
SCALE = 1.0 / np.sqrt(d)
GSC = 64.0                  # gate fp8 weight prescale

_cache = {}


def _build():
    import concourse.bacc as bacc
    import concourse.mybir as mybir
    from concourse.tile import TileContext

    f32 = mybir.dt.float32
    f32r = mybir.dt.float32r
    bf16 = mybir.dt.bfloat16
    f8 = mybir.dt.float8e4
    AF = mybir.ActivationFunctionType
    DR = mybir.MatmulPerfMode.DoubleRow

    def r_(ap):
        return ap.bitcast(f32r)

    nc = bacc.Bacc("TRN2", target_bir_lowering=False, debug=False,
                   num_devices=N_CORES)

    x_in = nc.dram_tensor("x", [ROWS, D], bf16, kind="ExternalInput").ap()
    w_in = nc.dram_tensor("w_qkv6", [D, BF_CT * 128], bf16,
                          kind="ExternalInput").ap()
    w8g_in = nc.dram_tensor("w8g", [128, HPC * 8 * 2 * 128], f8,
                            kind="ExternalInput").ap()
    wout_in = nc.dram_tensor("w_out", [D, D], bf16, kind="ExternalInput").ap()
    bg_in = nc.dram_tensor("b_gate", [128, HPC], f32, kind="ExternalInput").ap()
    cos_in = nc.dram_tensor("costab", [128, T], f32, kind="ExternalInput").ap()
    sin_in = nc.dram_tensor("sintab", [128, T], f32, kind="ExternalInput").ap()
    mask_in = nc.dram_tensor("m01", [128, 4 * QCH], bf16, kind="ExternalInput").ap()
    out_ext = nc.dram_tensor("out", [RC, D], bf16, kind="ExternalOutput").ap()

    with TileContext(nc) as tc:
        with tc.tile_pool(name="persist", bufs=1) as persist, \
             tc.tile_pool(name="dram", bufs=1, space="DRAM") as dram:
            a2a_in = [dram.tile([N_CORES * 128, RC], bf16, name=f"a2a_in{h}")
                      for h in range(HPC)]
            a2a_out = [dram.tile([N_CORES * 128, RC], bf16, name=f"a2a_out{h}")
                       for h in range(HPC)]

            ones_sq = persist.tile([128, 128], bf16, tag="ones_sq")
            ones_sqf = persist.tile([128, 128], f32, tag="ones_sqf")
            eps_col = persist.tile([128, 1], f32, tag="eps")
            bg_sb = persist.tile([128, HPC], f32, tag="bg")
            mask_sb = persist.tile([128, 4 * QCH], bf16, tag="mask")
            cc_sb = persist.tile([128, T], f32, tag="cc")
            ss_sb = persist.tile([128, T], f32, tag="ss")
            qT = [[persist.tile([128, T], bf16, tag=f"qT{h}{b}", name=f"qT{h}{b}")
                   for b in range(B)] for h in range(HPC)]
            kT = [[persist.tile([128, T], bf16, tag=f"kT{h}{b}", name=f"kT{h}{b}")
                   for b in range(B)] for h in range(HPC)]
            gT = [[persist.tile([128, T], bf16, tag=f"gT{h}{b}", name=f"gT{h}{b}")
                   for b in range(B)] for h in range(HPC)]
            v_sb = [persist.tile([128, ROWS], bf16, tag=f"v{h}", name=f"v_sb{h}")
                    for h in range(HPC)]

            # ---------------- Phase 1 ----------------
            with tc.tile_pool(name="wq", bufs=1) as wq, \
                 tc.tile_pool(name="p1", bufs=2) as p1, \
                 tc.tile_pool(name="p1xt", bufs=2) as p1xt, \
                 tc.tile_pool(name="p1x8", bufs=2) as p1x8, \
                 tc.tile_pool(name="pp_pj", bufs=6, space="PSUM") as pp_pj, \
                 tc.tile_pool(name="pp_st", bufs=2, space="PSUM") as pp_st:
                # chunk-0 x transpose first (sync queue owns all xbar
                # transposes: each fans out over all 16 DMA engines, two in
                # flight interleave and corrupt); weights on the scalar
                # HWDGE queue so they don't delay the transpose.
                xt0 = p1xt.tile([128, KC, RC], bf16, tag="xt", name="xt_0")
                nc.sync.dma_start(out=xt0[:], in_=x_in[0:RC, :], transpose=True)
                # w_sb[p, k, ct*128+c] = W[k*128+p, ct*128+c]; 4 k-group DMAs
                w_sb = wq.tile([128, KC, BF_CT * 128], bf16, tag="w")
                for k in range(KC):
                    nc.scalar.dma_start(
                        out=w_sb[:, k, :],
                        in_=w_in[k * 128:(k + 1) * 128, :])
                w8g_flat = wq.tile([128, HPC * 8 * 2 * 128], f8, tag="w8g")
                nc.scalar.dma_start(out=w8g_flat[:], in_=w8g_in[:])
                w8g_sb = w8g_flat[:].rearrange("p (h k j m) -> p h k j m",
                                               h=HPC, k=8, j=2)
                nc.scalar.dma_start(out=cc_sb[:], in_=cos_in[:])
                nc.scalar.dma_start(out=ss_sb[:], in_=sin_in[:])
                nc.scalar.dma_start(out=mask_sb[:], in_=mask_in[:])
                nc.scalar.dma_start(out=bg_sb[:], in_=bg_in[:])
                nc.vector.memset(ones_sq[:], 1.0)
                nc.vector.memset(ones_sqf[:], 1.0)
                nc.vector.memset(eps_col[:], EPS)

                for rc in range(NRC):
                    b = rc // 4
                    t0 = (rc % 4) * RC

                    if rc == 0:
                        xt = xt0
                    else:
                        xt = p1xt.tile([128, KC, RC], bf16, tag="xt",
                                       name=f"xt{rc}")
                        nc.sync.dma_start(
                            out=xt[:],
                            in_=x_in[rc * RC:(rc + 1) * RC, :],
                            transpose=True)
                    # paired-fp8 copy of xt for the gate DoubleRow matmuls
                    xt8 = p1x8.tile([128, 8, 2, RC], f8, tag="xt8",
                                    name=f"xt8_{rc}")
                    for k8 in range(8):
                        for j in range(2):
                            nc.vector.tensor_copy(xt8[:, k8, j, :],
                                                  xt[:, 2 * k8 + j, :])

                    # bf16 coltiles: 0=v0 1=v1 2=q0 3=q1 4=k0 5=k1
                    for ct in range(BF_CT):
                        h = ct % 2
                        ps = pp_pj.tile([128, RC], f32, tag="pj",
                                        name=f"pj{rc}_{ct}")
                        for k in range(KC):
                            nc.tensor.matmul(
                                ps[:],
                                w_sb[:, k, ct * 128:(ct + 1) * 128],
                                xt[:, k, :],
                                start=(k == 0), stop=(k == KC - 1))
                        if ct < 2:
                            # v: evict bf16, re-layout to natural via XBAR DMA
                            sv = p1.tile([128, RC], bf16, tag="sv")
                            nc.scalar.activation(sv[:], ps[:], AF.Copy)
                            vview = v_sb[h][:].rearrange(
                                "p (rt dd) -> p rt dd", dd=128)
                            nc.sync.dma_start(
                                out=vview[:, rc * 4:(rc + 1) * 4, :],
                                in_=sv[:], transpose=True)
                        else:
                            # q or k head: rmsnorm (all-ones f32r reduce
                            # pre-broadcasts across partitions) + rope
                            isq = ct < 4
                            sq = p1.tile([128, RC], f32r, tag="sq")
                            nc.scalar.activation(sq[:], ps[:], AF.Square)
                            ssq = pp_st.tile([128, RC], f32, tag="ssq")
                            nc.tensor.matmul(ssq[:], r_(ones_sqf[:]), sq[:],
                                             start=True, stop=True)
                            bc_sb = p1.tile([128, RC], f32, tag="bc_sb")
                            nc.scalar.activation(bc_sb[:], ssq[:],
                                                 AF.Abs_reciprocal_sqrt,
                                                 scale=1.0 / 128.0,
                                                 bias=eps_col[:])
                            qn = p1.tile([128, RC], f32, tag="qn")
                            nc.vector.tensor_mul(qn[:], ps[:], bc_sb[:])
                            # rope: fin = qn*cc + swap(qn)*ss; partition swap
                            # must be a copy (tensor_tensor ops need equal
                            # start partitions); cc=[cos;cos], ss=[-sin;sin]
                            sw = p1.tile([128, RC], f32, tag="sw")
                            nc.gpsimd.tensor_copy(sw[0:64, :], qn[64:128, :])
                            nc.gpsimd.tensor_copy(sw[64:128, :], qn[0:64, :])
                            nc.vector.tensor_mul(sw[:], sw[:], ss_sb[:, t0:t0 + RC])
                            nc.vector.tensor_mul(qn[:], qn[:], cc_sb[:, t0:t0 + RC])
                            park = qT if isq else kT
                            nc.vector.tensor_add(
                                park[h][b][:, t0:t0 + RC], qn[:], sw[:])
                    # gate coltiles in fp8 DoubleRow (weights x64 host-side)
                    for h in range(HPC):
                        ps = pp_pj.tile([128, RC], f32, tag="pj",
                                        name=f"pjg{rc}_{h}")
                        for k8 in range(8):
                            nc.tensor.matmul(
                                ps[:], w8g_sb[:, h, k8, :, :],
                                xt8[:, k8, :, :],
                                start=(k8 == 0), stop=(k8 == 7),
                                perf_mode=DR)
                        # park raw preactivation (x64); tanh sweep in phase 2
                        nc.scalar.activation(gT[h][b][:, t0:t0 + RC], ps[:],
                                             AF.Copy)

            # ---------------- Phase 2 (flat pipelined) + wout prefetch ----
            with tc.tile_pool(name="wout", bufs=1) as woutp:
                wout_tiles = {}
                for oc in range(4):
                    wt = woutp.tile([128, KC, 512], bf16, tag=f"wo{oc}",
                                    name=f"wo{oc}")
                    for k in range(KC):
                        nc.sync.dma_start(
                            out=wt[:, k, :],
                            in_=wout_in[k * 128:(k + 1) * 128,
                                        oc * 512:(oc + 1) * 512])
                    wout_tiles[oc] = wt
                # gat[p, k, r]: k even = head-0 dims (A2A#1), k odd = head-1
                # (A2A#2); loads issued right after each collective below
                gat = woutp.tile([128, KC, RC], bf16, tag="gat")

                PAIRS = ((0, 0), (0, 1), (1, 0), (1, 1))
                # flattened (h, b, qc, kt) score slots
                slots = []
                for h, bb in PAIRS:
                    for qc in range(T // QCH):
                        nkt = 4 * qc + 4
                        for kt in range(nkt):
                            slots.append((h, bb, qc, kt, nkt))
                NS = len(slots)

                with tc.tile_pool(name="p2", bufs=2) as p2, \
                     tc.tile_pool(name="p2e", bufs=6) as p2e, \
                     tc.tile_pool(name="pp_s", bufs=2, space="PSUM") as pp_s, \
                     tc.tile_pool(name="pp_o", bufs=2, space="PSUM") as pp_o, \
                     tc.tile_pool(name="pp_d", bufs=2, space="PSUM") as pp_d:
                    sc2 = {}      # pair index -> psum tile
                    ex2 = {}      # pair index -> sbuf exp tile
                    o_ps = {}     # (h,b,qc) -> pv psum tile
                    d_ps = {}     # (h,b,qc) -> denominator psum tile
                    swept = set()

                    def j0_of(qc, kt):
                        m = kt - 4 * qc
                        return 128 * m if m > 0 else 0

                    for i in range(NS + LOOK):
                        if i < NS:
                            h, bb, qc, kt, nkt = slots[i]
                            col0 = qc * QCH
                            qmv = qT[h][bb][:, col0:col0 + QCH]
                            j0 = j0_of(qc, kt)
                            half = kt % 2
                            pidx = i // 2
                            if half == 0:
                                sc2[pidx] = pp_s.tile([128, 2 * QCH], f32,
                                                      tag="sc",
                                                      name=f"sc{pidx}")
                            nc.tensor.matmul(
                                sc2[pidx][:, half * QCH + j0:(half + 1) * QCH],
                                kT[h][bb][:, kt * 128:(kt + 1) * 128],
                                qmv[:, j0:], start=True, stop=True)
                            if half == 1:
                                # exp over the whole pair; the region left of
                                # a diagonal j0 holds stale-but-finite psum,
                                # zeroed by the mask multiply below
                                j0e = j0_of(qc, kt - 1)
                                ex = p2e.tile([128, 2 * QCH], bf16, tag="ex",
                                              name=f"ex{pidx}")
                                nc.scalar.activation(ex[:, j0e:], sc2[pidx][:, j0e:],
                                                     AF.Exp, scale=SCALE)
                                m_e = (kt - 1) - 4 * qc
                                if m_e >= 0:
                                    nc.vector.tensor_mul(
                                        ex[:, j0e:], ex[:, j0e:],
                                        mask_sb[:, m_e * QCH + j0e:
                                                (m_e + 2) * QCH])
                                ex2[pidx] = ex
                                # tanh sweep for this pair's gate, under the
                                # exp table set, right after the first exp
                                if (h, bb) not in swept:
                                    swept.add((h, bb))
                                    nc.scalar.activation(
                                        gT[h][bb][:], gT[h][bb][:], AF.Tanh,
                                        scale=1.0 / (2.0 * GSC),
                                        bias=bg_sb[:, h:h + 1])
                        if i >= LOOK:
                            h, bb, qc, kt, nkt = slots[i - LOOK]
                            col0 = qc * QCH
                            j0 = j0_of(qc, kt)
                            half = kt % 2
                            pidx = (i - LOOK) // 2
                            key = (h, bb, qc)
                            if kt == 0:
                                o_ps[key] = pp_o.tile([128, QCH], f32, tag="o",
                                                      name=f"o{h}{bb}{qc}")
                                d_ps[key] = pp_d.tile([128, QCH], f32, tag="d",
                                                      name=f"d{h}{bb}{qc}")
                            nc.tensor.matmul(
                                o_ps[key][:, j0:],
                                v_sb[h][:, (bb * 16 + kt) * 128:
                                        (bb * 16 + kt + 1) * 128],
                                ex2[pidx][:, half * QCH + j0:(half + 1) * QCH],
                                start=(kt == 0), stop=(kt == nkt - 1),
                                skip_group_check=(j0 > 0))
                            # denominator accumulated on the PE (all-ones
                            # stationary pre-broadcasts across partitions)
                            nc.tensor.matmul(
                                d_ps[key][:, j0:], ones_sq[:],
                                ex2[pidx][:, half * QCH + j0:(half + 1) * QCH],
                                start=(kt == 0), stop=(kt == nkt - 1),
                                skip_group_check=(j0 > 0))
                            if kt == nkt - 1:
                                rec = p2.tile([128, QCH], f32, tag="rec")
                                nc.vector.reciprocal_approx_fast(
                                    rec[:], d_ps.pop(key)[:])
                                nm = p2.tile([128, QCH], f32, tag="nm")
                                nc.vector.tensor_mul(nm[:], o_ps.pop(key)[:],
                                                     rec[:])
                                # gate: sigmoid(z) = (1+tanh(z/2))/2, the 0.5
                                # lives in W_out; stt: (gT + 1) * nm
                                on_sb = p2.tile([128, QCH], bf16, tag="onsb")
                                nc.vector.scalar_tensor_tensor(
                                    out=on_sb[:],
                                    in0=gT[h][bb][:, col0:col0 + QCH],
                                    scalar=1.0,
                                    in1=nm[:],
                                    op0=mybir.AluOpType.add,
                                    op1=mybir.AluOpType.mult)
                                shard = bb * 4 + qc
                                nc.sync.dma_start(
                                    out=a2a_in[h][shard * 128:(shard + 1) * 128, :],
                                    in_=on_sb[:])
                                if bb == 1 and qc == T // QCH - 1:
                                    nc.gpsimd.collective_compute(
                                        "AllToAll", mybir.AluOpType.bypass,
                                        replica_groups=[list(range(N_CORES))],
                                        ins=[a2a_in[h].opt()],
                                        outs=[a2a_out[h].opt()])
                                    for kk in range(8):
                                        nc.sync.dma_start(
                                            out=gat[:, 2 * kk + h, :],
                                            in_=a2a_out[h][kk * 128:
                                                           (kk + 1) * 128, :])

                # ---------------- Phase 3: output projection ----------------
                with tc.tile_pool(name="p3", bufs=1) as p3, \
                     tc.tile_pool(name="p3e", bufs=3) as p3e, \
                     tc.tile_pool(name="pp_3", bufs=8, space="PSUM") as pp_3:
                    # pass A: even k (ready after A2A#1) -> SBUF partials
                    partials = {}
                    for oc in range(4):
                        for rt in range(4):
                            ps = pp_3.tile([128, 512], f32, tag="o3",
                                           name=f"psA_{oc}_{rt}")
                            for k in range(0, KC, 2):
                                nc.tensor.matmul(
                                    ps[:], gat[:, k, rt * 128:(rt + 1) * 128],
                                    wout_tiles[oc][:, k, :],
                                    start=(k == 0), stop=(k == KC - 2))
                            pa = p3.tile([128, 512], bf16, tag=f"pa{oc}{rt}",
                                         name=f"pa_{oc}_{rt}")
                            nc.vector.tensor_copy(pa[:], ps[:])
                            partials[(oc, rt)] = pa
                    # pass B: odd k (after A2A#2), add partial at evict
                    for oc in range(4):
                        for rt in range(4):
                            ps = pp_3.tile([128, 512], f32, tag="o3",
                                           name=f"psB_{oc}_{rt}")
                            for k in range(1, KC, 2):
                                nc.tensor.matmul(
                                    ps[:], gat[:, k, rt * 128:(rt + 1) * 128],
                                    wout_tiles[oc][:, k, :],
                                    start=(k == 1), stop=(k == KC - 1))
                            ev = p3e.tile([128, 512], bf16, tag="ev")
                            nc.vector.tensor_add(ev[:], ps[:], partials[(oc, rt)][:])
                            nc.sync.dma_start(
                                out=out_ext[rt * 128:(rt + 1) * 128,
                                            oc * 512:(oc + 1) * 512],
                                in_=ev[:])

    nc.compile()
    return nc


def _tables():
    inv = 1.0 / (ROPE_BASE ** (np.arange(0, d, 2, dtype=np.float64) / d))
    pos = np.arange(T, dtype=np.float64)
    ang = pos[None, :] * inv[:, None]          # [64, T]
    cos = np.cos(ang).astype(np.float32)
    sin = np.sin(ang).astype(np.float32)
    cc = np.concatenate([cos, cos], axis=0)    # [128, T]
    ss = np.concatenate([-sin, sin], axis=0)   # [128, T]
    return cc, ss


def kernel(x, W_qkv, W_out, W_gate, b_gate, mask):
    from concourse.bass_utils import run_bass_kernel_spmd
    import ml_dtypes

    if 'nc' not in _cache:
        _cache['nc'] = _build()
    nc = _cache['nc']

    x = np.ascontiguousarray(
        np.asarray(x, dtype=np.float32).reshape(ROWS, D)).astype(ml_dtypes.bfloat16)
    W_qkv = np.asarray(W_qkv, dtype=np.float32)
    W_out = np.ascontiguousarray(
        0.5 * np.asarray(W_out, dtype=np.float32)).astype(ml_dtypes.bfloat16)
    W_gate = np.asarray(W_gate, dtype=np.float32)
    b_gate = np.asarray(b_gate, dtype=np.float32)
    cos, sin = _tables()
    f = np.arange(QCH)[None, :]
    p = np.arange(128)[:, None]
    m01 = np.concatenate(
        [np.where(f >= p + 128 * m, 1.0, 0.0) for m in range(4)],
        axis=1).astype(ml_dtypes.bfloat16)

    in_maps = []
    for c in range(N_CORES):
        h0 = HPC * c
        cols = []
        for kind in (2, 0, 1):                    # v, q, k coltiles
            for h in range(h0, h0 + HPC):
                cols.append(W_qkv[:, kind * D + h * d:(kind * D + (h + 1) * d)])
        w_qkv6 = np.ascontiguousarray(
            np.concatenate(cols, axis=1)).astype(ml_dtypes.bfloat16)
        # gate fp8 DoubleRow tiles: w8g[p, h, k8, j, m] =
        #   GSC * W_gate[(2*k8+j)*128 + p, (h0+h)*128 + m]
        wg = W_gate[:, h0 * d:(h0 + HPC) * d] * GSC       # [2048, 256]
        w8 = wg.reshape(8, 2, 128, HPC, 128)              # [k8, j, p, h, m]
        w8 = w8.transpose(2, 3, 0, 1, 4).reshape(128, HPC * 8 * 2 * 128)
        w8g = np.ascontiguousarray(w8).astype(ml_dtypes.float8_e4m3)
        bg = np.ascontiguousarray(
            0.5 * b_gate[h0 * d:(h0 + HPC) * d].reshape(HPC, 128).T)
        in_maps.append({
            "x": x, "w_qkv6": w_qkv6, "w8g": w8g, "w_out": W_out,
            "b_gate": bg, "costab": cos, "sintab": sin, "m01": m01,
        })

    res = run_bass_kernel_spmd(nc, in_maps, list(range(N_CORES)))
    _cache['last_results'] = res
    out = np.concatenate(
        [np.asarray(res.results[c]["out"]).astype(np.float32)
         for c in range(N_CORES)], axis=0)
    return out.reshape(B, T, D)
